# revision 44
# baseline (speedup 1.0000x reference)
"""2D Gaussian splat rasterizer on 8 Trainium2 NeuronCores.

Strategy: the 256x256 image is split into 64 32x32 tiles, snake-dealt
8-per-core by descending gaussian count so the SPMD slot profile is
near-identical across cores. Gaussians are culled host-side by the exact
min-Mahalanobis distance between tile rect and gaussian (<= raster_ratio).
Each core's 8 tiles form one packed gaussian stream (slot k padded to the
max count over cores at that slot), cut into chunks of 128 (PE partition
dim). Per chunk:

    arg   = coefT.T @ basis        TensorE, K=6 float32r: -0.5*mahal2 +
                                   ln(opacity) in the 6-term pixel basis
                                   [x^2, xy, y^2, x, y, 1] (tile-local
                                   coords for accuracy; ln(opacity) folded
                                   into the constant-term coefficient)
    w     = Exp(arg)               ScalarE, PSUM src, fp16 out (no cutoff
                                   mask: the 3-sigma tail it adds is bounded
                                   ~6e-3 relative, inside the 2e-2 gate)
    per tile-segment of the chunk:
    out[tile] += colors.T @ w      TensorE, K=seg rows, fp16, PSUM accumulate
                                   into [35, 512] (row-halves at partition
                                   offsets 0/32, one PSUM bank)

Tile accumulators are copied (fp16, DVE) into one staging tile and written
back with two DMAs (one per row-half). Full [H, W, 3] image is reassembled
host-side (pure concatenation; no collectives).
"""

import numpy as np
import concourse.bacc as bacc
import concourse.tile as tile
from concourse import mybir
from concourse.bass_utils import run_bass_kernel_spmd

_runner_cache = {}


def _get_runner(nc):
    """Persistent jitted SPMD executor for a compiled Bass program (modeled on
    bass2jax.run_bass_via_pjrt's multi-core path, but cached so repeat calls
    reuse the same XLA executable — no retrace, no NEFF reload)."""
    key = id(nc)
    if key in _runner_cache:
        return _runner_cache[key]
    import jax
    import jax.numpy as jnp
    from jax.sharding import Mesh, PartitionSpec
    from jax.experimental.shard_map import shard_map
    from concourse import bass2jax, mybir as mb

    bass2jax.install_neuronx_cc_hook()

    in_names, out_names, out_avals, zero_outs = [], [], [], []
    partition_name = nc.partition_id_tensor.name if nc.partition_id_tensor else None
    for alloc in nc.m.functions[0].allocations:
        if not isinstance(alloc, mb.MemoryLocationSet):
            continue
        name = alloc.memorylocations[0].name
        if alloc.kind == "ExternalInput":
            if name != partition_name:
                in_names.append(name)
        elif alloc.kind == "ExternalOutput":
            shape = tuple(alloc.tensor_shape)
            dtype = mb.dt.np(alloc.dtype)
            out_names.append(name)
            out_avals.append(jax.core.ShapedArray(shape, dtype))
            zero_outs.append(np.zeros(shape, dtype))
    n_params = len(in_names)
    all_in = in_names + out_names + ([partition_name] if partition_name else [])

    def _body(*args):
        operands = list(args)
        if partition_name is not None:
            operands.append(bass2jax.partition_id_tensor())
        outs = bass2jax._bass_exec_p.bind(
            *operands,
            out_avals=tuple(out_avals),
            in_names=tuple(all_in),
            out_names=tuple(out_names),
            lowering_input_output_aliases=(),
            sim_require_finite=True,
            sim_require_nnan=True,
            nc=nc,
        )
        return tuple(outs)

    devices = jax.devices()[:N_CORES]
    mesh = Mesh(np.asarray(devices), ("core",))
    in_specs = (PartitionSpec("core"),) * (n_params + len(out_names))
    out_specs = (PartitionSpec("core"),) * len(out_names)
    sharded = jax.jit(
        shard_map(
            _body, mesh=mesh, in_specs=in_specs, out_specs=out_specs, check_rep=False
        ),
        donate_argnums=tuple(range(n_params, n_params + len(out_names))),
        keep_unused=True,
    )

    dev_in_cache = {}

    def run(in_maps, reuse_inputs=False):
        if reuse_inputs and "in" in dev_in_cache:
            concat_in = dev_in_cache["in"]
        else:
            concat_in = [
                np.concatenate([np.asarray(m[nm]) for m in in_maps], axis=0)
                for nm in in_names
            ]
            if reuse_inputs:
                from jax.sharding import NamedSharding

                sh = NamedSharding(mesh, PartitionSpec("core"))
                concat_in = [jax.device_put(a, sh) for a in concat_in]
                for a in concat_in:
                    a.block_until_ready()
                dev_in_cache["in"] = concat_in
        concat_zeros = [
            np.zeros((N_CORES * z.shape[0], *z.shape[1:]), z.dtype) for z in zero_outs
        ]
        out_arrs = sharded(*concat_in, *concat_zeros)
        out_arrs = [a.block_until_ready() for a in out_arrs]
        return [
            {
                nm: np.asarray(out_arrs[i]).reshape(N_CORES, *out_avals[i].shape)[c]
                for i, nm in enumerate(out_names)
            }
            for c in range(N_CORES)
        ]

    def time_loop(in_maps, n_calls):
        """Per-call wall times with inputs and donated zero-outputs pre-staged
        on device; outputs stay on device (only block_until_ready)."""
        import time as _t
        from jax.sharding import NamedSharding

        sh = NamedSharding(mesh, PartitionSpec("core"))
        concat_in = [
            jax.device_put(
                np.concatenate([np.asarray(m[nm]) for m in in_maps], axis=0), sh
            )
            for nm in in_names
        ]
        zeros_sets = [
            [
                jax.device_put(
                    np.zeros((N_CORES * z.shape[0], *z.shape[1:]), z.dtype), sh
                )
                for z in zero_outs
            ]
            for _ in range(n_calls)
        ]
        for a in concat_in:
            a.block_until_ready()
        for zs in zeros_sets:
            for a in zs:
                a.block_until_ready()
        # warm once (executable load)
        outs = sharded(*concat_in, *zeros_sets[0])
        [a.block_until_ready() for a in outs]
        times = []
        for i in range(1, n_calls):
            t0 = _t.perf_counter()
            outs = sharded(*concat_in, *zeros_sets[i])
            [a.block_until_ready() for a in outs]
            times.append(_t.perf_counter() - t0)
        return times

    def stage(in_maps, n_calls):
        """Pre-stage inputs + n_calls sets of donated zeros; return a closure
        that executes once per call (device exec + block)."""
        from jax.sharding import NamedSharding

        sh = NamedSharding(mesh, PartitionSpec("core"))
        concat_in = [
            jax.device_put(
                np.concatenate([np.asarray(m[nm]) for m in in_maps], axis=0), sh
            )
            for nm in in_names
        ]
        zeros_sets = [
            [
                jax.device_put(
                    np.zeros((N_CORES * z.shape[0], *z.shape[1:]), z.dtype), sh
                )
                for z in zero_outs
            ]
            for _ in range(n_calls)
        ]
        for a in concat_in:
            a.block_until_ready()
        for zs in zeros_sets:
            for a in zs:
                a.block_until_ready()
        state = {"i": 0}

        def call():
            i = state["i"]
            state["i"] += 1
            outs = sharded(*concat_in, *zeros_sets[i])
            # force full materialization — under the axon proxy,
            # block_until_ready alone does not wait for device execution
            return [np.asarray(a) for a in outs]

        return call

    def stage_async(in_maps, n_calls):
        """Like stage() but returns call(block=False) that does not wait."""
        from jax.sharding import NamedSharding

        sh = NamedSharding(mesh, PartitionSpec("core"))
        concat_in = [
            jax.device_put(
                np.concatenate([np.asarray(m[nm]) for m in in_maps], axis=0), sh
            )
            for nm in in_names
        ]
        zeros_sets = [
            [
                jax.device_put(
                    np.zeros((N_CORES * z.shape[0], *z.shape[1:]), z.dtype), sh
                )
                for z in zero_outs
            ]
            for _ in range(n_calls)
        ]
        for a in concat_in:
            a.block_until_ready()
        for zs in zeros_sets:
            for a in zs:
                a.block_until_ready()
        state = {"i": 0}

        def call(block=False):
            i = state["i"]
            state["i"] += 1
            outs = sharded(*concat_in, *zeros_sets[i])
            if block:
                outs = [np.asarray(a) for a in outs]
            return outs

        return call

    run.time_loop = time_loop
    run.stage = stage
    run.stage_async = stage_async
    _runner_cache[key] = run
    return run

N_CORES = 8
K = 6
STRIP_ROWS = 32
TILE_COLS = 32
F = STRIP_ROWS * TILE_COLS  # pixels per tile
QROWS = 4  # output row-quarter groups: out_ps is [99, F//QROWS] (1 PSUM bank)
FQ = F // QROWS

_prog_cache = {}
_Q_LC = "sp"
_Q_CBT = "sp"
_Q_OUT1 = "sp"


def _valid_seg(r0, r1):
    """PE tile_position row constraint: row offset must be quadrant-legal
    for the segment's row count."""
    n = r1 - r0
    if r0 == 0:
        return True
    if r0 == 64:
        return n <= 64
    if r0 in (32, 96):
        return n <= 32
    return False


def _split_seg(r0, r1):
    """Split [r0, r1) at quadrant boundaries until every piece is legal."""
    if _valid_seg(r0, r1):
        return [(r0, r1)]
    for cut in (64, 32, 96):
        if r0 < cut < r1:
            return _split_seg(r0, cut) + _split_seg(cut, r1)
    raise AssertionError((r0, r1))


def _schedule(caps):
    """Cut the padded gaussian stream (slot k occupies caps[k] positions,
    caps are multiples of 32) into chunks of 128; return (n_chunks,
    segments) where segments is a list of (chunk, r0, r1, slot, first,
    last)."""
    n_slots = len(caps)
    starts = np.concatenate([[0], np.cumsum(caps)])
    total = int(starts[-1])
    n_chunks = (total + 127) // 128
    segments = []
    for s in range(n_slots):
        lo, hi = int(starts[s]), int(starts[s + 1])
        segs = []
        p = lo
        while p < hi:
            j = p // 128
            q = min(hi, (j + 1) * 128)
            for r0, r1 in _split_seg(p - j * 128, q - j * 128):
                segs.append((j, r0, r1, s))
            p = q
        for i, seg in enumerate(segs):
            segments.append(seg + (i == 0, i == len(segs) - 1))
    segments.sort(key=lambda t: (t[0], t[1]))
    # PSUM liveness: how many slot accumulators are concurrently alive
    alive, max_alive = set(), 0
    for seg in segments:
        alive.add(seg[3])
        max_alive = max(max_alive, len(alive))
        if seg[5]:
            alive.discard(seg[3])
    return n_chunks, segments, max_alive


def _build_program(caps, repeat=1, ablate=""):
    """One SPMD program for the chunked gaussian stream described by caps.

    Inputs per core (3 DMAs):
      cbh [6, F + 128]      float32r: pixel basis + chunk-0 coefficients
      cbt [6, (C-1)*128]    float32r: remaining coefficients
      lc  [128, 4*C] fp32:  per-chunk ln(opacity) columns then per-chunk
                            colors (cast to fp16 on device once).
    Output: out [12, n_slots*FQ] fp16, one [12, FQ] block per tile slot
    (4 row-quarters x 3 channels).
    """
    n_slots = len(caps)
    C, segments, max_alive = _schedule(caps)
    assert max_alive <= 4, max_alive
    out_bufs = 4
    arg_bufs = 2
    nc = bacc.Bacc(
        "TRN2",
        target_bir_lowering=False,
        debug=False,
        enable_asserts=True,
        num_devices=N_CORES,
    )
    f32, f16, f32r = mybir.dt.float32, mybir.dt.float16, mybir.dt.float32r
    if "fp32" in ablate:
        f32r = f32
    cbh_ext = nc.dram_tensor("cbh", [K, F + 128], f32r, kind="ExternalInput").ap()
    cbt_ext = nc.dram_tensor(
        "cbt", [K, max(C - 1, 1) * 128], f32r, kind="ExternalInput"
    ).ap()
    lc_ext = nc.dram_tensor("lc", [128, 3 * C], f32, kind="ExternalInput").ap()
    out_ext = nc.dram_tensor("out", [3 * QROWS, n_slots * FQ], f16, kind="ExternalOutput").ap()

    # per chunk: list of its segments
    by_chunk = [[] for _ in range(C)]
    for seg in segments:
        by_chunk[seg[0]].append(seg)

    with tile.TileContext(nc) as tc:
        with (
            tc.tile_pool(name="consts", bufs=1) as consts,
            tc.tile_pool(name="work", bufs=3) as work,
            tc.tile_pool(name="psum", bufs=arg_bufs, space="PSUM") as psum,
            tc.tile_pool(name="psum_out", bufs=out_bufs, space="PSUM") as psum_out,
        ):
            # hoisted ACT exp-table warmup: no data deps, runs at t=0
            warm = consts.tile([1, 8], f32)
            nc.gpsimd.memset(warm[:], -1.0)
            warm16 = consts.tile([1, 8], f16)
            nc.scalar.activation(
                warm16[:], warm[:], mybir.ActivationFunctionType.Exp,
                bias=0.0, scale=1.0,
            )


            cbh_sb = consts.tile([K, F + 128], f32r)
            nc.sync.dma_start(out=cbh_sb[:], in_=cbh_ext[:])
            qmap = {"sp": nc.sync, "act": nc.scalar, "gps": nc.gpsimd}
            lc_sb = consts.tile([128, 3 * C], f32)
            qmap[_Q_LC].dma_start(out=lc_sb[:], in_=lc_ext[:])
            cbt_sb = consts.tile([K, max(C - 1, 1) * 128], f32r)
            qmap[_Q_CBT].dma_start(out=cbt_sb[:], in_=cbt_ext[:])
            col16 = consts.tile([128, 3 * C], f16)
            nc.vector.tensor_copy(col16[:], lc_sb[:])
            out_sb = consts.tile([99, n_slots * FQ], f16)

            out_ps = [None] * n_slots
            for rep in range(repeat):
                for j in range(C):
                    if j == 0:
                        lhsT = cbh_sb[0:K, F : F + 128]
                    else:
                        lhsT = cbt_sb[0:K, (j - 1) * 128 : j * 128]
                    arg_ps = psum.tile([128, F], f32, tag="arg")
                    for h in range(0, F, 512):
                        nc.tensor.matmul(
                            arg_ps[:, h : h + 512],
                            lhsT=lhsT,
                            rhs=cbh_sb[0:K, h : h + 512],
                            start=True,
                            stop=True,
                        )
                    w_sb = work.tile([128, F], f16, tag="w")
                    if "f16in" in ablate or ("mix" in ablate and j % 2 == 0):
                        a16 = work.tile([128, F], f16, tag="a16")
                        nc.vector.tensor_copy(a16[:], arg_ps[:])
                        nc.scalar.activation(
                            w_sb[:], a16[:], mybir.ActivationFunctionType.Exp,
                            bias=0.0, scale=1.0,
                        )
                    else:
                        nc.scalar.activation(
                            w_sb[:], arg_ps[:], mybir.ActivationFunctionType.Exp,
                            bias=0.0, scale=1.0,
                        )
                    for (cj, r0, r1, s, sfirst, slast) in by_chunk[j]:
                        if sfirst:
                            out_ps[s] = psum_out.tile(
                                [99, FQ], f32, tag="out", name=f"outps{s}_{rep}"
                            )
                        for q in range(QROWS):
                            nc.tensor.matmul(
                                out_ps[s][32 * q : 32 * q + 3, :],
                                lhsT=col16[r0:r1, j * 3 : j * 3 + 3],
                                rhs=w_sb[r0:r1, q * FQ : (q + 1) * FQ],
                                start=sfirst,
                                stop=slast,
                                tile_position=(r0, 32 * q),
                            )
                        if slast:
                            if "mix" in ablate and s % 2 == 0:
                                nc.scalar.copy(
                                    out_sb[:, s * FQ : (s + 1) * FQ], out_ps[s][:]
                                )
                            else:
                                nc.vector.tensor_copy(
                                    out_sb[:, s * FQ : (s + 1) * FQ], out_ps[s][:]
                                )
            for q in range(QROWS):
                eng = nc.sync if q == 0 else qmap[_Q_OUT1]
                eng.dma_start(
                    out=out_ext[3 * q : 3 * q + 3, :],
                    in_=out_sb[32 * q : 32 * q + 3, :],
                )
    nc.compile()
    return nc


def _get_program(caps, cutoff, repeat=1, ablate=""):
    key = (tuple(caps), float(cutoff), repeat, ablate)
    if key not in _prog_cache:
        _prog_cache[key] = _build_program(caps, repeat, ablate)
    return _prog_cache[key]


def _coefs(means, stds, rhos, cxo, cyo):
    """[6, G] coefficients of -0.5*mahal2 in local coords; f64 intermediates."""
    sx = stds[:, 0].astype(np.float64)
    sy = stds[:, 1].astype(np.float64)
    r = rhos.astype(np.float64)
    om = 1.0 - r * r
    ia = 1.0 / (sx * sx * om)
    ib = -r / (sx * sy * om)
    ic = 1.0 / (sy * sy * om)
    mxl = means[:, 0].astype(np.float64) - cxo
    myl = means[:, 1].astype(np.float64) - cyo
    return np.stack(
        [
            -0.5 * ia,
            -ib,
            -0.5 * ic,
            ia * mxl + ib * myl,
            ib * mxl + ic * myl,
            -0.5 * (ia * mxl * mxl + 2 * ib * mxl * myl + ic * myl * myl),
        ],
        axis=0,
    ).astype(np.float32)


def _basis(cxo_off=TILE_COLS / 2, cyo_off=STRIP_ROWS / 2):
    ys = np.arange(STRIP_ROWS, dtype=np.float64) + 0.5 - cyo_off
    xs = np.arange(TILE_COLS, dtype=np.float64) + 0.5 - cxo_off
    yl = np.repeat(ys, TILE_COLS)
    xl = np.tile(xs, STRIP_ROWS)
    return np.stack(
        [xl * xl, xl * yl, yl * yl, xl, yl, np.ones_like(xl)], axis=0
    ).astype(np.float32)


def kernel(
    opacity,
    means,
    stds,
    rhos,
    colors,
    image_height,
    image_width,
    scale_factor,
    raster_ratio,
    _repeat=1,
    _time_exec=False,
    _bench_calls=0,
    _ablate="",
):
    H = int(image_height)
    W = int(image_width)
    sf = float(scale_factor)
    rr = float(raster_ratio)
    opacity = np.asarray(opacity, np.float32)
    means = np.asarray(means, np.float32)
    stds = np.asarray(stds, np.float32) * np.float32(sf)
    rhos = np.asarray(rhos, np.float32)
    colors = np.asarray(colors, np.float32)
    N = opacity.shape[0]

    n_tiles_y = H // STRIP_ROWS
    n_tiles_x = W // TILE_COLS
    n_tiles = n_tiles_y * n_tiles_x
    assert n_tiles % N_CORES == 0
    n_slots = n_tiles // N_CORES

    # --- host-side cull: exact min Mahalanobis distance tile-rect vs gaussian
    mx = means[:, 0].astype(np.float64)
    my = means[:, 1].astype(np.float64)
    sx = stds[:, 0].astype(np.float64)
    sy = stds[:, 1].astype(np.float64)
    r64 = rhos.astype(np.float64)
    om = 1.0 - r64 * r64
    qa = 1.0 / (sx * sx * om)
    qc = 1.0 / (sy * sy * om)
    qb = -r64 / (sx * sy * om)

    def min_mahal2_rect(x0, x1, y0, y1):
        dx0 = x0 - mx
        dx1 = x1 - mx
        dy0 = y0 - my
        dy1 = y1 - my
        inside = (dx0 <= 0) & (dx1 >= 0) & (dy0 <= 0) & (dy1 >= 0)
        best = np.where(inside, 0.0, np.inf)
        for dx in (dx0, dx1):
            dys = np.clip(-qb * dx / qc, dy0, dy1)
            best = np.minimum(best, qa * dx * dx + 2 * qb * dx * dys + qc * dys * dys)
        for dy in (dy0, dy1):
            dxs = np.clip(-qb * dy / qa, dx0, dx1)
            best = np.minimum(best, qa * dxs * dxs + 2 * qb * dxs * dy + qc * dy * dy)
        return best

    tile_ids = []  # per tile: gaussian index array
    tile_pos = []  # per tile: (ty, tx) pixel origin
    for tyi in range(n_tiles_y):
        ty = tyi * STRIP_ROWS
        for txi in range(n_tiles_x):
            tx = txi * TILE_COLS
            m2 = min_mahal2_rect(
                tx + 0.5, tx + TILE_COLS - 0.5, ty + 0.5, ty + STRIP_ROWS - 0.5
            )
            m = m2 <= rr * rr + 1e-9
            tile_ids.append(np.nonzero(m)[0])
            tile_pos.append((ty, tx))

    # snake-deal tiles to cores by descending gaussian count, so every core
    # gets a near-identical sorted profile (SPMD: slot capacity is the max
    # over cores at each slot position)
    gcnt = [len(ids) for ids in tile_ids]
    t_order = sorted(range(n_tiles), key=lambda t: -gcnt[t])
    assign = [[] for _ in range(N_CORES)]
    for i, t in enumerate(t_order):
        rnd, pos = divmod(i, N_CORES)
        core = pos if rnd % 2 == 0 else N_CORES - 1 - pos
        assign[core].append(t)
    caps = tuple(
        max(32, (max(gcnt[assign[core][k]] for core in range(N_CORES)) + 31) // 32 * 32)
        for k in range(n_slots)
    )
    starts = np.concatenate([[0], np.cumsum(caps)]).astype(int)
    C = (int(starts[-1]) + 127) // 128

    cutoff = -0.5 * rr * rr
    nc = _get_program(caps, cutoff, _repeat, _ablate)

    basis = _basis()  # [6, F]
    lnop_all = np.where(
        opacity > 0, np.log(np.maximum(opacity, 1e-45)), -1e4
    ).astype(np.float32)

    in_maps = []
    perms = []  # per core: slot -> (ty, tx)
    for core in range(N_CORES):
        coef_stream = np.zeros((K, C * 128), np.float32)
        coef_stream[5, :] = -1e4
        lc_arr = np.zeros((128, 3 * C), np.float32)
        perm = []
        for k in range(n_slots):
            t = assign[core][k]
            ty, tx = tile_pos[t]
            perm.append((ty, tx))
            ids = tile_ids[t]
            g = len(ids)
            assert g <= caps[k]
            if g:
                cxo = tx + TILE_COLS / 2
                cyo = ty + STRIP_ROWS / 2
                p0 = int(starts[k])
                cf = _coefs(means[ids], stds[ids], rhos[ids], cxo, cyo)
                cf[5] += lnop_all[ids]
                coef_stream[:, p0 : p0 + g] = cf
                ln = lnop_all[ids]
                col = colors[ids]
                # scatter into [128, C] chunk-column layout
                pos = p0 + np.arange(g)
                cj = pos // 128
                rr_ = pos % 128
                for ch in range(3):
                    lc_arr[rr_, cj * 3 + ch] = col[:, ch]
        perms.append(perm)
        cbh = np.concatenate([basis, coef_stream[:, :128]], axis=1)
        cbt = coef_stream[:, 128:] if C > 1 else np.zeros((K, 128), np.float32)
        in_maps.append({"cbh": cbh, "cbt": cbt, "lc": lc_arr})

    import time as _time

    global _last_in_maps
    _last_in_maps = in_maps
    run = _get_runner(nc)
    if _bench_calls:
        return run.time_loop(in_maps, _bench_calls)
    t0 = _time.time()
    results = run(in_maps, reuse_inputs=_time_exec)
    exec_wall = _time.time() - t0

    out = np.zeros((H, W, 3), np.float32)
    hq = STRIP_ROWS // QROWS
    for core in range(N_CORES):
        o = np.asarray(results[core]["out"], np.float32)  # [12, n_slots*FQ]
        for k, (ty, tx) in enumerate(perms[core]):
            blk = o[:, k * FQ : (k + 1) * FQ]  # [12, FQ]
            for q in range(QROWS):
                sub = blk[3 * q : 3 * q + 3, :].reshape(3, hq, TILE_COLS)
                out[ty + q * hq : ty + (q + 1) * hq, tx : tx + TILE_COLS, :] = (
                    sub.transpose(1, 2, 0)
                )
    if _time_exec:
        return out, exec_wall
    return out
